# revision 3
# baseline (speedup 1.0000x reference)
"""Multi-head attention kernel for 8 Trainium2 NeuronCores.

Problem: embeddings [4, 2048, 1024], 16 heads x 64 dim, torch nn.Linear
convention (x @ W.T + b) for Q/K/V/O projections.

Sharding: batch (4) x head-halves (2) -> 8 cores. Core c handles batch
c//2, local heads (c%2)*8..(c%2)*8+8. Output projection is row-sharded;
host sums the two partial outputs per batch element and adds
bo' = bo + bv @ wo.T (V-bias folded on host).

Per-core dataflow (feature dims on partitions; PE is the critical
engine -- matmul streaming is ~80% of the critical path):
  xT [1024e, 2048t] bf16 (host pre-transposed + cast)
  QT/KT [(h,d)=512, t] via PE, bias added on DVE during PSUM evac.
  V [t, (h,d)] via PE (no bias -- folded into bo on host).
  Per head-quad (4 heads = 2 pairs), per q-block of 512, per k-tile:
    scores_T[k,q] row-paired matmuls (2 heads share the PE array),
    exp on ScalarE (1/8 scale folded in, no max subtraction needed),
    U[(2x64),q] col-paired matmuls, sumexp via 4 col-tiled M=1
    ones-matmuls into one PSUM bank (partitions 0/32/64/96).
  normalize: recip(sumexp) -> gpsimd partition-broadcast -> DVE mult
  (the mults are deferred ~2 steps so the boundary latency chain does
  not block the in-order DVE queue).
  yT[e_out, t] = woT.T @ attn_T accumulated over 4 pair-tiles.
Projection/outproj groups are spread by an EDF scheduler so the PE
never idles long enough for HAM to re-throttle the clock; junk
matmuls warm the PE at t=0 and bridge the epilogue.
Host: out[b] = (yT[2b] + yT[2b+1]).T + bo'.
"""

import sys

sys.path.insert(0, "/opt/trn_rl_repo")

import numpy as np
import ml_dtypes

import concourse.bass as bass
import concourse.bacc as bacc
import concourse.mybir as mybir
import concourse.tile as tile
from concourse.bass_utils import run_bass_kernel_spmd

BF16 = mybir.dt.bfloat16
F32 = mybir.dt.float32
NPBF16 = ml_dtypes.bfloat16

B, S, E = 4, 2048, 1024
H_LOC = 8          # local heads per core
D = 64             # head dim
OL = H_LOC * D     # 512 local output dim
N_CORES = 8
QB = 512           # query block (free dim of scores_T)
NQB = S // QB      # 4
NKT = S // 128     # 16 key tiles
NET = E // 128     # 8 embed tiles


def build_program():
    from contextlib import ExitStack

    nc = bacc.Bacc("TRN2", debug=False, num_devices=N_CORES)

    xT = nc.dram_tensor("xT", [E, S], BF16, kind="ExternalInput")
    wqT = nc.dram_tensor("wqT", [E, OL], BF16, kind="ExternalInput")
    wkT = nc.dram_tensor("wkT", [E, OL], BF16, kind="ExternalInput")
    wvT = nc.dram_tensor("wvT", [E, OL], BF16, kind="ExternalInput")
    woT = nc.dram_tensor("woT", [OL, E], BF16, kind="ExternalInput")
    bqc = nc.dram_tensor("bqc", [128, 4], F32, kind="ExternalInput")
    bkc = nc.dram_tensor("bkc", [128, 4], F32, kind="ExternalInput")
    yT = nc.dram_tensor("yT", [E, S], F32, kind="ExternalOutput")

    with tile.TileContext(nc) as tc, ExitStack() as est:
        xt_p = est.enter_context(tc.tile_pool(name="xt", bufs=NET))
        wq_p = est.enter_context(tc.tile_pool(name="wq", bufs=NET))
        wk_p = est.enter_context(tc.tile_pool(name="wk", bufs=NET))
        wv_p = est.enter_context(tc.tile_pool(name="wv", bufs=NET))
        wo_p = est.enter_context(tc.tile_pool(name="wo", bufs=4))
        bias_p = est.enter_context(tc.tile_pool(name="bias", bufs=4))
        qt_p = est.enter_context(tc.tile_pool(name="qt", bufs=4))
        kt_p = est.enter_context(tc.tile_pool(name="kt", bufs=4))
        vb_p = est.enter_context(tc.tile_pool(name="vb", bufs=NKT))
        pj_p = est.enter_context(tc.tile_pool(name="pj", bufs=1, space="PSUM"))
        sc_p = est.enter_context(tc.tile_pool(name="sc", bufs=2, space="PSUM"))
        u_p = est.enter_context(tc.tile_pool(name="u", bufs=2, space="PSUM"))
        se_p = est.enter_context(tc.tile_pool(name="se", bufs=1, space="PSUM"))
        ex_p = est.enter_context(tc.tile_pool(name="ex", bufs=20))
        at_p = est.enter_context(tc.tile_pool(name="at", bufs=16))
        nrm_p = est.enter_context(tc.tile_pool(name="nrm", bufs=4))
        ys_p = est.enter_context(tc.tile_pool(name="ys", bufs=2))
        usb_p = est.enter_context(tc.tile_pool(name="usb", bufs=4))

        # ---- load inputs ----
        # Preload the exp activation table while input DMAs run.
        warm = bias_p.tile([1, 16], F32, tag="warm")
        nc.vector.memset(warm[:], 0.0)
        warm2 = bias_p.tile([1, 16], F32, tag="warm2")
        nc.scalar.activation(warm2[:], warm[:],
                             mybir.ActivationFunctionType.Exp)
        xts = [xt_p.tile([128, S], BF16, tag="xt", name="xt")
               for _ in range(NET)]
        wts = {
            name: [pool.tile([128, OL], BF16, tag="w" + name,
                             name="w" + name) for _ in range(NET)]
            for name, pool in (("q", wq_p), ("k", wk_p), ("v", wv_p))
        }
        # interleave so the first projection groups' inputs land first
        for e in range(NET):
            nc.sync.dma_start(xts[e][:], xT[e * 128:(e + 1) * 128, :])
            nc.sync.dma_start(wts["q"][e][:], wqT[e * 128:(e + 1) * 128, :])
            nc.sync.dma_start(wts["k"][e][:], wkT[e * 128:(e + 1) * 128, :])
            nc.sync.dma_start(wts["v"][e][:], wvT[e * 128:(e + 1) * 128, :])
        wos = [wo_p.tile([128, E], BF16, tag="wo", name="wo")
               for _ in range(4)]
        bqs = bias_p.tile([128, 4], F32, tag="bqc")
        bks = bias_p.tile([128, 4], F32, tag="bkc")
        ones = bias_p.tile([1, 128], BF16, tag="ones")
        onecol = bias_p.tile([128, 1], BF16, tag="onecol")
        junkm = bias_p.tile([128, QB], BF16, tag="junkm")
        nc.sync.dma_start(bqs[:], bqc[:])
        nc.sync.dma_start(bks[:], bkc[:])
        nc.vector.memset(ones[:], 1.0)
        nc.vector.memset(onecol[:], 1.0)
        nc.vector.memset(junkm[:], 0.0)

        # ---- PE warm-up: junk matmuls so HAM un-throttles the clock
        # while input DMAs land (nothing depends on these).
        def junk_burst(n, tag):
            jk = se_p.tile([128, QB], F32, tag="se", name=tag)
            for _ in range(n):
                nc.tensor.matmul(
                    jk[0:1, :], onecol[:], junkm[:],
                    start=True, stop=True, skip_group_check=True,
                ).annotate("junk")

        junk_burst(10, "warmjk")

        qts = [qt_p.tile([128, S], BF16, tag="qt", name="qt")
               for _ in range(4)]
        kts = [kt_p.tile([128, S], BF16, tag="kt", name="kt")
               for _ in range(4)]
        vbs = [vb_p.tile([128, OL], BF16, tag="vb", name="vb")
               for _ in range(NKT)]
        atts = [[at_p.tile([128, QB], BF16, tag="at", name="at")
                 for _ in range(4)] for _ in range(NQB)]

        # ---- projection / outproj group emitters (PE fillers) ----
        def qk_group(i, j, which):
            """Q or K projection for o-tile i, t-block j (one PSUM group)."""
            w = wts[which]
            bias_t = bqs if which == "q" else bks
            dest = qts[i] if which == "q" else kts[i]
            acc = pj_p.tile([128, QB], F32, tag="pj", name="pj")
            for e in range(NET):
                nc.tensor.matmul(
                    acc[:],
                    w[e][:, i * 128:(i + 1) * 128],
                    xts[e][:, j * QB:(j + 1) * QB],
                    start=(e == 0), stop=(e == NET - 1),
                ).annotate("qkp")
            nc.vector.tensor_scalar_add(
                dest[:, j * QB:(j + 1) * QB], acc[:], bias_t[:, i:i + 1])

        def v_group(ti):
            acc = pj_p.tile([128, OL], F32, tag="pj", name="pjv")
            for e in range(NET):
                nc.tensor.matmul(
                    acc[:],
                    xts[e][:, ti * 128:(ti + 1) * 128],
                    wts["v"][e][:],
                    start=(e == 0), stop=(e == NET - 1),
                ).annotate("vp")
            nc.vector.tensor_copy(vbs[ti][:], acc[:])

        def outproj_group(qb, eo, pool=None, tag="pj"):
            y = (pool or pj_p).tile([128, QB], F32, tag=tag, name="y")
            for p2 in range(4):
                nc.tensor.matmul(
                    y[:],
                    wos[p2][:, eo * 128:(eo + 1) * 128],
                    atts[qb][p2][:],
                    start=(p2 == 0), stop=(p2 == 3),
                ).annotate("op")
            ysb = ys_p.tile([128, QB], F32, tag="ys", name="ys")
            nc.vector.tensor_copy(ysb[:], y[:])
            nc.sync.dma_start(
                yT[eo * 128:(eo + 1) * 128, qb * QB:(qb + 1) * QB], ysb[:])

        # ---- filler schedule (EDF) ----
        def qg(i, j):
            return lambda: qk_group(i, j, "q")

        def kg(i, j):
            return lambda: qk_group(i, j, "k")

        def vg(t):
            return lambda: v_group(t)

        def og(qb, e):
            return lambda: outproj_group(qb, e)

        # Flat software-pipelined step list: one step per (quad, qb,
        # pair-in-quad, kt). At step i the ScalarE exp for step i is
        # emitted first, then the scores matmuls for step i+1, then PE
        # filler groups, then the U / sumexp matmuls for step i (which
        # wait on exp i) -- so ScalarE always has its next input queued.
        step_list = []
        for quad in range(2):
            for qb in range(NQB):
                for pi in range(2):
                    for kt in range(NKT):
                        step_list.append((quad, qb, 2 * quad + pi, pi, kt))
        nsteps = len(step_list)

        def sidx(quad, qb, pi, kt):
            return ((quad * NQB + qb) * 2 + pi) * NKT + kt

        prologue = [qg(0, 0), kg(0, 0), vg(0), vg(1), vg(2)]

        # EDF items: (earliest, deadline, thunk). Deadlines carry a
        # 2-step margin for the DVE evac of the group's PSUM.
        items = []
        for t in range(3, NKT):
            items.append((0, t - 2, vg(t)))
        for p in range(4):
            for j in range(4):
                if (p, j) == (0, 0):
                    continue
                items.append((0, sidx(p // 2, 0, p % 2, 4 * j) - 2, kg(p, j)))
        for p in range(4):
            for j in range(4):
                if (p, j) == (0, 0):
                    continue
                items.append((0, sidx(p // 2, j, p % 2, 0) - 2, qg(p, j)))
        # outproj for qb becomes available once quad1/qb's normalize
        # (deferred mults land ~2 steps into the next block) is done.
        for qb in range(3):
            for e in range(NET):
                items.append((sidx(1, qb, 1, 15) + 4 + e, nsteps - 3,
                              og(qb, e)))

        sched = {}
        pending = sorted(items, key=lambda it: (it[0], it[1]))
        active = []
        pi_idx = 0
        for s in range(nsteps):
            while pi_idx < len(pending) and pending[pi_idx][0] <= s:
                active.append(pending[pi_idx])
                pi_idx += 1
            active.sort(key=lambda it: it[1])
            placed = 0
            while active and placed < 2:
                if placed == 1 and active[0][1] > s + 2:
                    break   # keep slack unless the next item is nearly due
                it = active.pop(0)
                sched.setdefault(s, []).append(it[2])
                placed += 1
        for it in active:   # leftovers (shouldn't happen)
            sched.setdefault(nsteps - 4, []).append(it[2])

        for th in prologue:
            th()
        for p in range(4):
            nc.sync.dma_start(wos[p][:], woT[p * 128:(p + 1) * 128, :])

        # ---- attention ----
        def emit_scores(quad, qb, pair, kt):
            sc = sc_p.tile([128, 2 * QB], F32, tag="sc", name="sc")
            nc.tensor.matmul(
                sc[:, 0:QB],
                kts[pair][0:64, kt * 128:(kt + 1) * 128],
                qts[pair][0:64, qb * QB:(qb + 1) * QB],
                start=True, stop=True, tile_position=(0, 0),
            ).annotate("scA")
            nc.tensor.matmul(
                sc[:, QB:2 * QB],
                kts[pair][64:128, kt * 128:(kt + 1) * 128],
                qts[pair][64:128, qb * QB:(qb + 1) * QB],
                start=True, stop=True, tile_position=(64, 0),
            ).annotate("scB")
            return sc

        def emit_normalize_mults(st):
            """The 4 atts mults for a finished (quad, qb) block."""
            qb = st["qb"]
            for u2, pr in ((st["uA_sb"], st["pA"]), (st["uB_sb"], st["pB"])):
                for sub in range(2):
                    g = (pr % 2) * 2 + sub
                    nc.vector.tensor_mul(
                        atts[qb][pr][sub * 64:(sub + 1) * 64, :],
                        u2[sub * 64:(sub + 1) * 64, :],
                        st["bcf"][g][sub * 64:(sub + 1) * 64, :])

        q0, q1, p1, _, k1 = step_list[0]
        pend_sc = emit_scores(q0, q1, p1, k1)
        cur = {}      # per-(quad,qb) state: uA, uB, seb, etA list
        done_norm = None   # finished block awaiting deferred mults
        for i, (quad, qb, pair, pi, kt) in enumerate(step_list):
            if (pi, kt) == (0, 0):
                cur["uA"] = u_p.tile([128, QB], F32, tag="u", name="uA")
                cur["uB"] = u_p.tile([128, QB], F32, tag="u", name="uB")
                cur["seb"] = se_p.tile([128, QB], F32, tag="se", name="seb")
                cur["etA"] = [None] * NKT
            # deferred normalize mults from the previous block
            if done_norm is not None and kt == 2:
                emit_normalize_mults(done_norm)
                done_norm = None
            # exp for this step
            et = ex_p.tile([128, 2 * QB], BF16, tag="ex", name="ex")
            nc.scalar.activation(
                et[:], pend_sc[:],
                mybir.ActivationFunctionType.Exp, scale=0.125).annotate("exp")
            if pi == 0:
                cur["etA"][kt] = et
            # scores for next step
            if i + 1 < nsteps:
                nq, nqb, npair, _, nkt = step_list[i + 1]
                pend_sc = emit_scores(nq, nqb, npair, nkt)
            # fillers
            for th in sched.get(i, []):
                th()
            # U matmuls for this step
            u = cur["uA"] if pi == 0 else cur["uB"]
            for sub in range(2):
                hcol = (pair * 2 + sub) * D
                nc.tensor.matmul(
                    u[sub * 64:(sub + 1) * 64, :],
                    vbs[kt][:, hcol:hcol + D],
                    et[:, sub * QB:(sub + 1) * QB],
                    start=(kt == 0), stop=(kt == NKT - 1),
                    tile_position=(0, sub * 64),
                    skip_group_check=True,
                ).annotate(f"u{sub}")
            if pi == 1 and kt == 0:
                # pair A's U is complete: evacuate it so its PSUM bank
                # frees long before the next q-block needs it
                ua_sb = usb_p.tile([128, QB], BF16, tag="usb", name="ua_sb")
                nc.vector.tensor_copy(ua_sb[:], cur["uA"][:])
                cur["uA_sb"] = ua_sb
            if pi == 1:
                # quad-packed sumexp: 4 col-tiled M=1 matmuls, one bank
                seb = cur["seb"]
                epair = (cur["etA"][kt], et)
                for g in range(4):
                    nc.tensor.matmul(
                        seb[g * 32:g * 32 + 1, :],
                        onecol[:],
                        epair[g // 2][:, (g % 2) * QB:(g % 2 + 1) * QB],
                        start=(kt == 0), stop=(kt == NKT - 1),
                        tile_position=(0, g * 32),
                        skip_group_check=True,
                    ).annotate(f"se{g}")
                if kt == NKT - 1:
                    # ---- evacuate B + sumexp; defer the mults ----
                    pA, pB = 2 * quad, 2 * quad + 1
                    ub_sb = usb_p.tile([128, QB], BF16, tag="usb",
                                       name="ub_sb")
                    nc.vector.tensor_copy(ub_sb[:], cur["uB"][:])
                    bcfs = {}
                    for g in range(4):
                        rcs = nrm_p.tile([1, QB], F32, tag="rcs",
                                         name="rcs")
                        nc.vector.tensor_copy(
                            rcs[:], cur["seb"][g * 32:g * 32 + 1, :])
                        rcr = nrm_p.tile([1, QB], F32, tag="rcr",
                                         name="rcr")
                        nc.vector.reciprocal_approx_fast(rcr[:], rcs[:])
                        bcf = nrm_p.tile([128, QB], F32, tag="bcf",
                                         name="bcf")
                        nc.gpsimd.partition_broadcast(bcf[:], rcr[:])
                        bcfs[g] = bcf
                    st = {"qb": qb, "pA": pA, "pB": pB,
                          "uA_sb": cur["uA_sb"], "uB_sb": ub_sb,
                          "bcf": bcfs}
                    if i == nsteps - 1:
                        emit_normalize_mults(st)
                    else:
                        done_norm = st
        # tail: bridge the normalize latency so HAM stays warm, then the
        # last q-block's output projection (scores pool is free by now --
        # use its banks so the groups pipeline)
        junk_burst(8, "tailjk")
        for eo in range(NET):
            outproj_group(3, eo, pool=sc_p, tag="sc")

    nc.compile()
    return nc


_CACHED = {}


def _get_program():
    if "nc" not in _CACHED:
        _CACHED["nc"] = build_program()
    return _CACHED["nc"]


def make_inputs(embeddings, wq, bq, wk, bk, wv, bv, wo, bo):
    """Host-side sharding: per-core input maps."""
    in_maps = []
    for c in range(N_CORES):
        b, half = c // 2, c % 2
        sl = slice(half * OL, (half + 1) * OL)
        in_maps.append({
            "xT": np.ascontiguousarray(embeddings[b].T).astype(NPBF16),
            "wqT": np.ascontiguousarray(wq[sl, :].T).astype(NPBF16),
            "wkT": np.ascontiguousarray(wk[sl, :].T).astype(NPBF16),
            "wvT": np.ascontiguousarray(wv[sl, :].T).astype(NPBF16),
            "woT": np.ascontiguousarray(wo[:, sl].T).astype(NPBF16),
            "bqc": np.ascontiguousarray(
                bq[sl].reshape(4, 128).T).astype(np.float32),
            "bkc": np.ascontiguousarray(
                bk[sl].reshape(4, 128).T).astype(np.float32),
        })
    return in_maps


def unshard(results, bo_eff):
    out = np.empty((B, S, E), np.float32)
    for b in range(B):
        yt = results[2 * b]["yT"] + results[2 * b + 1]["yT"]
        out[b] = yt.T + bo_eff[None, :]
    return out


def kernel(embeddings, wq, bq, wk, bk, wv, bv, wo, bo, _trace=False):
    embeddings = np.asarray(embeddings, np.float32)
    wo = np.asarray(wo, np.float32)
    bv = np.asarray(bv, np.float32)
    bo_eff = np.asarray(bo, np.float32) + bv @ wo.T
    nc = _get_program()
    in_maps = make_inputs(
        embeddings, np.asarray(wq, np.float32), np.asarray(bq, np.float32),
        np.asarray(wk, np.float32), np.asarray(bk, np.float32),
        np.asarray(wv, np.float32), bv, wo, np.asarray(bo, np.float32))
    res = run_bass_kernel_spmd(
        nc, in_maps, core_ids=list(range(N_CORES)), trace=_trace)
    out = unshard(res.results, bo_eff)
    if _trace:
        kernel.last_result = res
    return out


# revision 12
# speedup vs baseline: 1.0675x; 1.0675x over previous
"""Multi-head attention kernel for 8 Trainium2 NeuronCores.

Problem: embeddings [4, 2048, 1024], 16 heads x 64 dim, torch nn.Linear
convention (x @ W.T + b) for Q/K/V/O projections.

Sharding: batch (4) x head-halves (2) -> 8 cores. Core c handles batch
c//2, local heads (c%2)*8..(c%2)*8+8. Output projection is row-sharded;
host sums the two partial outputs per batch element and adds
bo' = bo + bv @ wo.T (V-bias folded on host).

Per-core dataflow (feature dims on partitions; PE is the critical
engine -- matmul streaming is ~80% of the critical path):
  xT [1024e, 2048t] bf16 (host pre-transposed + cast)
  QT/KT [(h,d)=512, t] via PE, bias added on DVE during PSUM evac.
  V [t, (h,d)] via PE (no bias -- folded into bo on host).
  Per head-quad (4 heads = 2 pairs), per q-block of 512, per k-tile:
    scores_T[k,q] row-paired matmuls (2 heads share the PE array),
    exp on ScalarE (1/8 scale folded in, no max subtraction needed),
    U[(2x64),q] col-paired matmuls, sumexp via 4 col-tiled M=1
    ones-matmuls into one PSUM bank (partitions 0/32/64/96).
  normalize: recip(sumexp) -> gpsimd partition-broadcast -> DVE mult
  (the mults are deferred ~2 steps so the boundary latency chain does
  not block the in-order DVE queue).
  yT[e_out, t] = woT.T @ attn_T accumulated over 4 pair-tiles.
Projection/outproj groups are spread by an EDF scheduler so the PE
never idles long enough for HAM to re-throttle the clock; junk
matmuls warm the PE at t=0 and bridge the epilogue.
Host: out[b] = (yT[2b] + yT[2b+1]).T + bo'.
"""

import sys

sys.path.insert(0, "/opt/trn_rl_repo")

import numpy as np
import ml_dtypes

import concourse.bass as bass
import concourse.bacc as bacc
import concourse.mybir as mybir
import concourse.tile as tile
from concourse.bass_utils import run_bass_kernel_spmd

BF16 = mybir.dt.bfloat16
F32 = mybir.dt.float32
NPBF16 = ml_dtypes.bfloat16

B, S, E = 4, 2048, 1024
H_LOC = 8          # local heads per core
D = 64             # head dim
OL = H_LOC * D     # 512 local output dim
N_CORES = 8
QB = 512           # query block (free dim of scores_T)
NQB = S // QB      # 4
NKT = S // 128     # 16 key tiles
NET = E // 128     # 8 embed tiles


def build_program():
    from contextlib import ExitStack

    nc = bacc.Bacc("TRN2", debug=False, num_devices=N_CORES)

    xT = nc.dram_tensor("xT", [E, S], BF16, kind="ExternalInput")
    wqT = nc.dram_tensor("wqT", [E, OL], BF16, kind="ExternalInput")
    wkT = nc.dram_tensor("wkT", [E, OL], BF16, kind="ExternalInput")
    wvT = nc.dram_tensor("wvT", [E, OL], BF16, kind="ExternalInput")
    woT = nc.dram_tensor("woT", [OL, E], BF16, kind="ExternalInput")
    bqc = nc.dram_tensor("bqc", [128, 4], F32, kind="ExternalInput")
    bkc = nc.dram_tensor("bkc", [128, 4], F32, kind="ExternalInput")
    yT = nc.dram_tensor("yT", [E, S], F32, kind="ExternalOutput")

    with tile.TileContext(nc) as tc, ExitStack() as est:
        xt_p = est.enter_context(tc.tile_pool(name="xt", bufs=NET))
        wq_p = est.enter_context(tc.tile_pool(name="wq", bufs=NET))
        wk_p = est.enter_context(tc.tile_pool(name="wk", bufs=NET))
        wv_p = est.enter_context(tc.tile_pool(name="wv", bufs=NET))
        wo_p = est.enter_context(tc.tile_pool(name="wo", bufs=4))
        bias_p = est.enter_context(tc.tile_pool(name="bias", bufs=4))
        qt_p = est.enter_context(tc.tile_pool(name="qt", bufs=4))
        kt_p = est.enter_context(tc.tile_pool(name="kt", bufs=4))
        vb_p = est.enter_context(tc.tile_pool(name="vb", bufs=NKT))
        pj_p = est.enter_context(tc.tile_pool(name="pj", bufs=1, space="PSUM"))
        sc_p = est.enter_context(tc.tile_pool(name="sc", bufs=2, space="PSUM"))
        u_p = est.enter_context(tc.tile_pool(name="u", bufs=2, space="PSUM"))
        se_p = est.enter_context(tc.tile_pool(name="se", bufs=1, space="PSUM"))
        ex_p = est.enter_context(tc.tile_pool(name="ex", bufs=20))
        at_p = est.enter_context(tc.tile_pool(name="at", bufs=16))
        nrm_p = est.enter_context(tc.tile_pool(name="nrm", bufs=4))
        ys_p = est.enter_context(tc.tile_pool(name="ys", bufs=2))
        usb_p = est.enter_context(tc.tile_pool(name="usb", bufs=4))

        # ---- load inputs ----
        # Preload the exp activation table while input DMAs run.
        warm = bias_p.tile([1, 16], F32, tag="warm")
        nc.vector.memset(warm[:], 0.0)
        warm2 = bias_p.tile([1, 16], F32, tag="warm2")
        nc.scalar.activation(warm2[:], warm[:],
                             mybir.ActivationFunctionType.Exp)
        xts = [xt_p.tile([128, S], BF16, tag="xt", name="xt")
               for _ in range(NET)]
        wts = {
            name: [pool.tile([128, OL], BF16, tag="w" + name,
                             name="w" + name) for _ in range(NET)]
            for name, pool in (("q", wq_p), ("k", wk_p), ("v", wv_p))
        }
        # DMA order: first attention block's inputs land first.  x is
        # column-blocked so q-block 0 (plus wq) arrives in ~2MB, letting
        # the main loop start ~12us in instead of waiting for all 8.4MB.
        for e in range(NET):
            nc.sync.dma_start(wts["q"][e][:], wqT[e * 128:(e + 1) * 128, :])
            nc.sync.dma_start(xts[e][:, 0:QB], xT[e * 128:(e + 1) * 128, 0:QB])
        for e in range(NET):
            nc.sync.dma_start(wts["k"][e][:], wkT[e * 128:(e + 1) * 128, :])
        for e in range(NET):
            nc.sync.dma_start(wts["v"][e][:], wvT[e * 128:(e + 1) * 128, :])
        for jb in range(1, 4):
            for e in range(NET):
                nc.sync.dma_start(
                    xts[e][:, jb * QB:(jb + 1) * QB],
                    xT[e * 128:(e + 1) * 128, jb * QB:(jb + 1) * QB])
        wos = [wo_p.tile([128, E], BF16, tag="wo", name="wo")
               for _ in range(4)]
        bqs = bias_p.tile([128, 4], F32, tag="bqc")
        bks = bias_p.tile([128, 4], F32, tag="bkc")
        ones = bias_p.tile([1, 128], BF16, tag="ones")
        onecol = bias_p.tile([128, 1], BF16, tag="onecol")
        junkm = bias_p.tile([128, QB], BF16, tag="junkm")
        nc.sync.dma_start(bqs[:], bqc[:])
        nc.sync.dma_start(bks[:], bkc[:])
        nc.vector.memset(ones[:], 1.0)
        nc.vector.memset(onecol[:], 1.0)
        nc.vector.memset(junkm[:], 0.0)

        # ---- PE warm-up: junk matmuls so HAM un-throttles the clock
        # while input DMAs land (nothing depends on these).
        def junk_burst(n, tag):
            jk = se_p.tile([128, QB], F32, tag="se", name=tag)
            for _ in range(n):
                nc.tensor.matmul(
                    jk[0:1, :], onecol[:], junkm[:],
                    start=True, stop=True, skip_group_check=True,
                ).annotate("junk")

        junk_burst(8, "warmjk")

        qts = [qt_p.tile([128, S], BF16, tag="qt", name="qt")
               for _ in range(4)]
        kts = [kt_p.tile([128, S], BF16, tag="kt", name="kt")
               for _ in range(4)]
        vbs = [vb_p.tile([128, OL], BF16, tag="vb", name="vb")
               for _ in range(NKT)]
        atts = [[at_p.tile([128, QB], BF16, tag="at", name="at")
                 for _ in range(4)] for _ in range(NQB)]

        # ---- projection / outproj group emitters (PE fillers) ----
        def qk_group(i, j, which, sprinkle=0):
            """Q or K projection for o-tile i, t-block j (one PSUM group).

            sprinkle: junk matmuls emitted after each e-step so the PE
            stays busy (HAM warm) while the group is DMA-gated.
            """
            w = wts[which]
            bias_t = bqs if which == "q" else bks
            dest = qts[i] if which == "q" else kts[i]
            acc = pj_p.tile([128, QB], F32, tag="pj", name="pj")
            jk = None
            if sprinkle:
                jk = se_p.tile([128, QB], F32, tag="se", name="sprjk")
            for e in range(NET):
                nc.tensor.matmul(
                    acc[:],
                    w[e][:, i * 128:(i + 1) * 128],
                    xts[e][:, j * QB:(j + 1) * QB],
                    start=(e == 0), stop=(e == NET - 1),
                ).annotate("qkp")
                for _ in range(sprinkle if e < NET - 1 else 0):
                    nc.tensor.matmul(
                        jk[64:65, :], onecol[:], junkm[:],
                        start=True, stop=True, skip_group_check=True,
                        tile_position=(0, 64),
                    ).annotate("junk")
            nc.vector.tensor_scalar_add(
                dest[:, j * QB:(j + 1) * QB], acc[:], bias_t[:, i:i + 1])

        def v_group(ti):
            acc = pj_p.tile([128, OL], F32, tag="pj", name="pjv")
            for e in range(NET):
                nc.tensor.matmul(
                    acc[:],
                    xts[e][:, ti * 128:(ti + 1) * 128],
                    wts["v"][e][:],
                    start=(e == 0), stop=(e == NET - 1),
                ).annotate("vp")
            nc.vector.tensor_copy(vbs[ti][:], acc[:])

        def outproj_group(qb, eo, pool=None, tag="pj"):
            y = (pool or pj_p).tile([128, QB], F32, tag=tag, name="y")
            for p2 in range(4):
                nc.tensor.matmul(
                    y[:],
                    wos[p2][:, eo * 128:(eo + 1) * 128],
                    atts[qb][p2][:],
                    start=(p2 == 0), stop=(p2 == 3),
                ).annotate("op")
            ysb = ys_p.tile([128, QB], F32, tag="ys", name="ys")
            nc.vector.tensor_copy(ysb[:], y[:])
            nc.sync.dma_start(
                yT[eo * 128:(eo + 1) * 128, qb * QB:(qb + 1) * QB], ysb[:])

        # ---- filler schedule (EDF) ----
        def qg(i, j):
            return lambda: qk_group(i, j, "q")

        def kg(i, j):
            return lambda: qk_group(i, j, "k")

        def vg(t):
            return lambda: v_group(t)

        def og(qb, e):
            return lambda: outproj_group(qb, e)

        # Flat software-pipelined step list: one step per (quad, qb,
        # pair-in-quad, kt). At step i the ScalarE exp for step i is
        # emitted first, then the scores matmuls for step i+1, then PE
        # filler groups, then the U / sumexp matmuls for step i (which
        # wait on exp i) -- so ScalarE always has its next input queued.
        step_list = []
        for quad in range(2):
            for qb in range(NQB):
                for pi in range(2):
                    for kt in range(NKT):
                        step_list.append((quad, qb, 2 * quad + pi, pi, kt))
        nsteps = len(step_list)

        def sidx(quad, qb, pi, kt):
            return ((quad * NQB + qb) * 2 + pi) * NKT + kt

        prologue = [lambda: qk_group(0, 0, "q", sprinkle=3),
                    lambda: qk_group(0, 0, "k", sprinkle=1),
                    vg(0), vg(1), vg(2)]

        # Filler items: (earliest, deadline, n_matmuls, thunk).
        # earliest reflects DMA arrival of the x column-block (~3 steps
        # per 2MB batch) and, for quad1's Q/K, a hold-back so quad1 is
        # not starved of fillers; deadline carries a 2-step margin for
        # the DVE evac of the group's PSUM.
        items = []
        for t in range(3, NKT):
            items.append((3 * (t // 4), t - 2, 8, vg(t)))
        for p in range(4):
            for j in range(4):
                if (p, j) == (0, 0):
                    continue
                d = sidx(p // 2, 0, p % 2, 4 * j) - 2
                items.append((max(3 * j, d - 24), d, 8, kg(p, j)))
        for p in range(4):
            for j in range(4):
                if (p, j) == (0, 0):
                    continue
                d = sidx(p // 2, j, p % 2, 0) - 2
                items.append((max(3 * j, d - 24), d, 8, qg(p, j)))
        # outproj for qb becomes available once quad1/qb's normalize
        # (deferred mults land ~2 steps into the next block) is done;
        # staggered deadlines spread the 8 groups across the next block.
        for qb in range(3):
            for e in range(NET):
                d = min(163 + 32 * qb + 4 * (e + 1), nsteps - 3)
                items.append((163 + 32 * qb, d, 4, og(qb, e)))

        # Uniform-budget placement: fillers flow at the average filler
        # rate so the PE never starves (HAM stays warm); deadlines are
        # enforced, earliest respected.
        items = [(min(e, d), d, m, th) for (e, d, m, th) in items]
        total_mm = sum(it[2] for it in items)
        rate = total_mm / nsteps
        sched = {}
        pending = sorted(items, key=lambda it: (it[0], it[1]))
        active = []
        pi_idx = 0
        budget = 2.0
        emitted = 0
        for s in range(nsteps):
            while pi_idx < len(pending) and pending[pi_idx][0] <= s:
                active.append(pending[pi_idx])
                pi_idx += 1
            active.sort(key=lambda it: it[1])
            budget += rate
            placed = 0
            while active and (placed < 2 or active[0][1] <= s):
                due = active[0][1] <= s + 2
                if not due and emitted + active[0][2] > budget:
                    break
                it = active.pop(0)
                sched.setdefault(s, []).append(it[3])
                emitted += it[2]
                placed += 1
        assert not active and pi_idx == len(pending), "unplaced fillers"


        for th in prologue:
            th()
        for p in range(4):
            nc.sync.dma_start(wos[p][:], woT[p * 128:(p + 1) * 128, :])

        # ---- attention ----
        def emit_scores(quad, qb, pair, kt):
            sc = sc_p.tile([128, 2 * QB], F32, tag="sc", name="sc")
            nc.tensor.matmul(
                sc[:, 0:QB],
                kts[pair][0:64, kt * 128:(kt + 1) * 128],
                qts[pair][0:64, qb * QB:(qb + 1) * QB],
                start=True, stop=True, tile_position=(0, 0),
            ).annotate("scA")
            nc.tensor.matmul(
                sc[:, QB:2 * QB],
                kts[pair][64:128, kt * 128:(kt + 1) * 128],
                qts[pair][64:128, qb * QB:(qb + 1) * QB],
                start=True, stop=True, tile_position=(64, 0),
            ).annotate("scB")
            return sc

        def emit_normalize_mults(st):
            """The 4 atts mults for a finished (quad, qb) block."""
            qb = st["qb"]
            for u2, pr in ((st["uA_sb"], st["pA"]), (st["uB_sb"], st["pB"])):
                for sub in range(2):
                    g = (pr % 2) * 2 + sub
                    nc.vector.tensor_mul(
                        atts[qb][pr][sub * 64:(sub + 1) * 64, :],
                        u2[sub * 64:(sub + 1) * 64, :],
                        st["bcf"][g][sub * 64:(sub + 1) * 64, :])

        q0, q1, p1, _, k1 = step_list[0]
        pend_sc = emit_scores(q0, q1, p1, k1)
        cur = {}      # per-(quad,qb) state: uA, uB, seb, etA list
        done_norm = None   # finished block awaiting deferred mults
        for i, (quad, qb, pair, pi, kt) in enumerate(step_list):
            if (pi, kt) == (0, 0):
                cur["uA"] = u_p.tile([128, QB], F32, tag="u", name="uA")
                cur["uB"] = u_p.tile([128, QB], F32, tag="u", name="uB")
                cur["seb"] = se_p.tile([128, QB], F32, tag="se", name="seb")
                cur["etA"] = [None] * NKT
            # deferred normalize mults from the previous block
            if done_norm is not None and kt == 2:
                emit_normalize_mults(done_norm)
                done_norm = None
            # exp for this step
            et = ex_p.tile([128, 2 * QB], BF16, tag="ex", name="ex")
            nc.scalar.activation(
                et[:], pend_sc[:],
                mybir.ActivationFunctionType.Exp, scale=0.125).annotate("exp")
            if pi == 0:
                cur["etA"][kt] = et
            # scores for next step
            if i + 1 < nsteps:
                nq, nqb, npair, _, nkt = step_list[i + 1]
                pend_sc = emit_scores(nq, nqb, npair, nkt)
            # fillers
            for th in sched.get(i, []):
                th()
            # U matmuls for this step
            u = cur["uA"] if pi == 0 else cur["uB"]
            for sub in range(2):
                hcol = (pair * 2 + sub) * D
                nc.tensor.matmul(
                    u[sub * 64:(sub + 1) * 64, :],
                    vbs[kt][:, hcol:hcol + D],
                    et[:, sub * QB:(sub + 1) * QB],
                    start=(kt == 0), stop=(kt == NKT - 1),
                    tile_position=(0, sub * 64),
                    skip_group_check=True,
                ).annotate(f"u{sub}")
            if pi == 1 and kt == 0:
                # pair A's U is complete: evacuate it so its PSUM bank
                # frees long before the next q-block needs it
                ua_sb = usb_p.tile([128, QB], BF16, tag="usb", name="ua_sb")
                nc.vector.tensor_copy(ua_sb[:], cur["uA"][:])
                cur["uA_sb"] = ua_sb
            if pi == 1:
                # quad-packed sumexp: 4 col-tiled M=1 matmuls, one bank
                seb = cur["seb"]
                epair = (cur["etA"][kt], et)
                for g in range(4):
                    nc.tensor.matmul(
                        seb[g * 32:g * 32 + 1, :],
                        onecol[:],
                        epair[g // 2][:, (g % 2) * QB:(g % 2 + 1) * QB],
                        start=(kt == 0), stop=(kt == NKT - 1),
                        tile_position=(0, g * 32),
                        skip_group_check=True,
                    ).annotate(f"se{g}")
                if kt == NKT - 1:
                    # ---- evacuate B + sumexp; defer the mults ----
                    pA, pB = 2 * quad, 2 * quad + 1
                    ub_sb = usb_p.tile([128, QB], BF16, tag="usb",
                                       name="ub_sb")
                    nc.vector.tensor_copy(ub_sb[:], cur["uB"][:])
                    bcfs = {}
                    for g in range(4):
                        rcs = nrm_p.tile([1, QB], F32, tag="rcs",
                                         name="rcs")
                        nc.vector.tensor_copy(
                            rcs[:], cur["seb"][g * 32:g * 32 + 1, :])
                        rcr = nrm_p.tile([1, QB], F32, tag="rcr",
                                         name="rcr")
                        nc.vector.reciprocal_approx_fast(rcr[:], rcs[:])
                        bcf = nrm_p.tile([128, QB], F32, tag="bcf",
                                         name="bcf")
                        nc.gpsimd.partition_broadcast(bcf[:], rcr[:])
                        bcfs[g] = bcf
                    st = {"qb": qb, "pA": pA, "pB": pB,
                          "uA_sb": cur["uA_sb"], "uB_sb": ub_sb,
                          "bcf": bcfs}
                    if i == nsteps - 1:
                        emit_normalize_mults(st)
                    else:
                        done_norm = st
        # tail: bridge the normalize latency so HAM stays warm, then the
        # last q-block's output projection (scores pool is free by now --
        # use its banks so the groups pipeline)
        junk_burst(14, "tailjk")
        for eo in range(NET):
            outproj_group(3, eo, pool=sc_p, tag="sc")

    nc.compile()
    return nc


_CACHED = {}


def _get_program():
    if "nc" not in _CACHED:
        _CACHED["nc"] = build_program()
    return _CACHED["nc"]


def make_inputs(embeddings, wq, bq, wk, bk, wv, bv, wo, bo):
    """Host-side sharding: per-core input maps."""
    in_maps = []
    for c in range(N_CORES):
        b, half = c // 2, c % 2
        sl = slice(half * OL, (half + 1) * OL)
        in_maps.append({
            "xT": np.ascontiguousarray(embeddings[b].T).astype(NPBF16),
            "wqT": np.ascontiguousarray(wq[sl, :].T).astype(NPBF16),
            "wkT": np.ascontiguousarray(wk[sl, :].T).astype(NPBF16),
            "wvT": np.ascontiguousarray(wv[sl, :].T).astype(NPBF16),
            "woT": np.ascontiguousarray(wo[:, sl].T).astype(NPBF16),
            "bqc": np.ascontiguousarray(
                bq[sl].reshape(4, 128).T).astype(np.float32),
            "bkc": np.ascontiguousarray(
                bk[sl].reshape(4, 128).T).astype(np.float32),
        })
    return in_maps


def unshard(results, bo_eff):
    out = np.empty((B, S, E), np.float32)
    for b in range(B):
        yt = results[2 * b]["yT"] + results[2 * b + 1]["yT"]
        out[b] = yt.T + bo_eff[None, :]
    return out


def kernel(embeddings, wq, bq, wk, bk, wv, bv, wo, bo, _trace=False):
    embeddings = np.asarray(embeddings, np.float32)
    wo = np.asarray(wo, np.float32)
    bv = np.asarray(bv, np.float32)
    bo_eff = np.asarray(bo, np.float32) + bv @ wo.T
    nc = _get_program()
    in_maps = make_inputs(
        embeddings, np.asarray(wq, np.float32), np.asarray(bq, np.float32),
        np.asarray(wk, np.float32), np.asarray(bk, np.float32),
        np.asarray(wv, np.float32), bv, wo, np.asarray(bo, np.float32))
    res = run_bass_kernel_spmd(
        nc, in_maps, core_ids=list(range(N_CORES)), trace=_trace)
    out = unshard(res.results, bo_eff)
    if _trace:
        kernel.last_result = res
    return out
